# revision 31
# baseline (speedup 1.0000x reference)
"""Trainium2 Bass kernel for nn_MoEFeedForward_11536282157172.

Strategy (data-parallel over tokens, sparse expert compute per core):
  - 8 cores, each owns 1024 tokens of the 8192 (B*T) tokens.
  - Per core, all on device:
      * gate scores (fp32 matmul) + sigmoid + group-restricted top-2 routing
        (vector ops), producing per-token combine weights for the 2 picked
        experts.
      * dispatch lists: for each expert, the (sorted) list of its token ids,
        built with a matmul-based exclusive cumsum over selection masks and
        a dma_scatter_add into a slot table.
      * per expert: dma_gather (transposed) of its tokens' activations
        (bf16), FFN silu(x@w1)@w2 on the tensor engine with fp32 PSUM
        accumulation, scale by the gathered combine weight, and
        dma_scatter_add of the rows into the output (CCE add in DMA).
      * shared expert computed densely for the core's 1024 tokens.
  - Host side only shards/concats/transposes/casts.
"""

import numpy as np
import ml_dtypes

import concourse.bass as bass
import concourse.mybir as mybir
import concourse.tile as tile
from concourse import bacc
from concourse.bass_utils import run_bass_kernel_spmd

BF16 = ml_dtypes.bfloat16
F32 = mybir.dt.float32
BF = mybir.dt.bfloat16
I16 = mybir.dt.int16

B, T, D, H, E = 4, 2048, 2048, 1408, 16
G, GS = 4, 4                     # routing groups, experts per group
NCORES = 8
NTOK = (B * T) // NCORES         # tokens per core
CAP = 256                        # per-expert per-core token capacity
KD = D // 128                    # 16 contraction chunks over D
KH = H // 128                    # 11 chunks over H
AF = mybir.ActivationFunctionType
OP = mybir.AluOpType
AX = mybir.AxisListType


def emit_moe(tc, ins, outs, ntok=NTOK, cap=CAP, dbg=False):
    """Emit the per-core MoE program. ins/outs: dicts of DRAM APs."""
    nc = tc.nc
    NT = ntok // 128             # token tiles
    CI = cap // 16               # idx columns per expert (wrap-16)
    MT = cap // 128              # M tiles per expert in matmul2

    xT = ins["xT"]               # [D, ntok] f32
    xTb = ins["xTb"]             # [D, ntok] bf16
    xr = ins["xr"]               # [ntok+1, D] bf16, last row zeros
    gwT = ins["gwT"]             # [D, E] f32
    w1 = ins["w1"]               # [E, KH, 128, KD, 128] bf16 (see _host_prep)
    w2 = ins["w2"]               # [E, H, D] bf16
    sw1 = ins["sw1"]             # [KH, 128, KD, 128] bf16 (see _host_prep)
    sw2 = ins["sw2"]             # [H, D] bf16
    ltri = ins["ltri"]           # [128,128] f32 strict lower triangular ones
    onesq = ins["ones"]          # [128,128] f32 ones
    ecrow = ins["ecrow"]         # [128, E] f32, every row = [0, cap, 2cap, ...]
    iota = ins["iota"]           # [128, 1] f32 = 0..127
    tokid = ins["tokid"]         # [128, NT] f32: tokid[p,t] = t*128+p+1
    out = outs["out"]            # [ntok+1, D] f32

    if dbg:
        combine = outs["combine"]   # [ntok+1, 64] f32
        listbuf = outs["listbuf"]   # [E*cap, 64] f32
    else:
        combine = nc.dram_tensor("combine_i", [ntok + 1, 64], F32).ap()
        listbuf = nc.dram_tensor("listbuf_i", [E * cap, 64], F32).ap()
    sbounce = nc.dram_tensor("sbounce_i", [2, ntok], I16).ap()

    with (
        tc.tile_pool(name="constp", bufs=1) as constp,
        tc.tile_pool(name="meta", bufs=1) as meta,
        tc.tile_pool(name="psA", bufs=2, space="PSUM") as psA,
    ):
        # ---- constants to SBUF ----
        ltri_c = constp.tile([128, 128], F32)
        nc.sync.dma_start(out=ltri_c[:], in_=ltri)
        ones_c = constp.tile([128, 128], F32)
        nc.sync.dma_start(out=ones_c[:], in_=onesq)
        ecrow_c = constp.tile([128, E], F32)
        nc.sync.dma_start(out=ecrow_c[:], in_=ecrow)
        iota_c = constp.tile([128, 1], F32)
        nc.sync.dma_start(out=iota_c[:], in_=iota)
        tokid_c = constp.tile([128, NT], F32)
        nc.sync.dma_start(out=tokid_c[:], in_=tokid)
        gw_c = constp.tile([128, KD, E], F32)
        nc.sync.dma_start(out=gw_c[:], in_=gwT.rearrange("(k p) e -> p k e", p=128))

        # zero tile for clearing DRAM scratch
        zt = meta.tile([128, 512], F32)
        nc.gpsimd.memset(zt[:], 0.0)
        # zero the slot table (scatter-add accumulates into it)
        lbv = listbuf.rearrange("(q p r) c -> q p (r c)", p=128, r=(E * cap) // (128 * 4))
        for q in range(4):
            nc.sync.dma_start(out=lbv[q], in_=zt[:, : lbv.shape[2]])
        # zero the combine-weight table (pad row must be zero; the rest keeps
        # the simulator's uninitialized-read check happy)
        cflat = combine.rearrange("a b -> (a b)")
        nc.sync.dma_start(
            out=cflat[0 : ntok * 64].rearrange("(p r) -> p r", p=128),
            in_=zt[:, : ntok // 2],
        )
        nc.sync.dma_start(
            out=cflat[ntok * 64 :].rearrange("(p r) -> p r", p=1), in_=zt[0:1, 0:64]
        )

        # ---- routing state (lives until expert phase ends) ----
        sel_all = meta.tile([128, NT, E], F32)
        is1_all = meta.tile([128, NT, E], F32)
        is2_all = meta.tile([128, NT, E], F32)
        w_all = meta.tile([128, NT, E], F32)
        slot1 = meta.tile([128, NT], F32)
        slot2 = meta.tile([128, NT], F32)
        gidx = meta.tile([128, E, CI], I16)
        nc.gpsimd.memset(gidx[:], 0)

        # shared count registers for the SWDGE custom ops (each to_reg of a
        # value > 255 would otherwise burn a fresh gpsimd register)
        ntok_reg = nc.gpsimd.to_reg(ntok)
        cap_reg = nc.gpsimd.to_reg(cap)

        # ---- phase A: gate matmul + per-tile routing math ----
        rt = ctx_rt = tc.alloc_tile_pool(name="rt", bufs=2)
        with tc.tile_pool(name="xtf", bufs=1) as xtfp:
            xtf = xtfp.tile([128, KD, ntok], F32)
            nc.sync.dma_start(out=xtf[:], in_=xT.rearrange("(k p) n -> p k n", p=128))

            for mt in range(NT):
                ps_g = psA.tile([128, E], F32, tag="psg")
                for k in range(KD):
                    nc.tensor.matmul(
                        ps_g[:],
                        xtf[:, k, mt * 128 : (mt + 1) * 128],
                        gw_c[:, k, :],
                        start=(k == 0),
                        stop=(k == KD - 1),
                    )
                s = rt.tile([128, E], F32, tag="s")
                nc.scalar.activation(s[:], ps_g[:], AF.Sigmoid)

                # group scores: sum of top-2 of each group of 4 = max pairwise sum
                s3 = s.rearrange("p (g u) -> p g u", u=GS)
                gsc = rt.tile([128, G], F32, tag="gsc")
                pair = rt.tile([128, G], F32, tag="pair")
                first = True
                for i in range(GS):
                    for j in range(i + 1, GS):
                        tgt = gsc if first else pair
                        nc.vector.tensor_tensor(tgt[:], s3[:, :, i], s3[:, :, j], OP.add)
                        if not first:
                            nc.vector.tensor_tensor(gsc[:], gsc[:], pair[:], OP.max)
                        first = False
                # 2nd largest of the 4 group scores = max of pairwise mins
                m2g = rt.tile([128, 1], F32, tag="m2g")
                pmin = rt.tile([128, 1], F32, tag="pmin")
                first = True
                for i in range(G):
                    for j in range(i + 1, G):
                        tgt = m2g if first else pmin
                        nc.vector.tensor_tensor(tgt[:], gsc[:, i : i + 1], gsc[:, j : j + 1], OP.min)
                        if not first:
                            nc.vector.tensor_tensor(m2g[:], m2g[:], pmin[:], OP.max)
                        first = False
                gmask = rt.tile([128, G], F32, tag="gmask")
                nc.vector.tensor_scalar(gmask[:], gsc[:], m2g[:], None, op0=OP.is_ge)

                # masked scores over all 16 experts
                masked = rt.tile([128, E], F32, tag="masked")
                gm3 = gmask.unsqueeze(-1).broadcast_to([128, G, GS])
                nc.vector.tensor_tensor(
                    masked.rearrange("p (g u) -> p g u", u=GS), s3, gm3, OP.mult
                )

                # top-2 of masked
                m1 = rt.tile([128, 1], F32, tag="m1")
                nc.vector.tensor_reduce(m1[:], masked[:], axis=AX.X, op=OP.max)
                is1 = is1_all[:, mt, :]
                nc.vector.tensor_scalar(is1, masked[:], m1[:], None, op0=OP.is_ge)
                nis1 = rt.tile([128, E], F32, tag="nis1")  # 1 - is1
                nc.vector.tensor_scalar(nis1[:], is1, -1.0, 1.0, op0=OP.mult, op1=OP.add)
                masked2 = rt.tile([128, E], F32, tag="masked2")
                nc.vector.tensor_tensor(masked2[:], masked[:], nis1[:], OP.mult)
                m2 = rt.tile([128, 1], F32, tag="m2")
                nc.vector.tensor_reduce(m2[:], masked2[:], axis=AX.X, op=OP.max)
                is2 = is2_all[:, mt, :]
                nc.vector.tensor_scalar(is2, masked2[:], m2[:], None, op0=OP.is_ge)
                sel = sel_all[:, mt, :]
                nc.vector.tensor_tensor(sel, is1, is2, OP.add)

                # combine weights = masked*sel / (m1+m2+eps)
                den = rt.tile([128, 1], F32, tag="den")
                nc.vector.tensor_tensor(den[:], m1[:], m2[:], OP.add)
                nc.vector.tensor_scalar_add(den[:], den[:], 1e-20)
                rden = rt.tile([128, 1], F32, tag="rden")
                nc.vector.reciprocal(rden[:], den[:])
                wraw = rt.tile([128, E], F32, tag="wraw")
                nc.vector.tensor_tensor(wraw[:], masked[:], sel, OP.mult)
                nc.vector.tensor_scalar(w_all[:, mt, :], wraw[:], rden[:], None, op0=OP.mult)

                # write combine-weight rows to DRAM for later gathering
                nc.sync.dma_start(
                    out=combine[mt * 128 : (mt + 1) * 128, 0:E], in_=w_all[:, mt, :]
                )

            # ---- exclusive cumsum of sel over the global token order ----
            for mt in range(NT):
                ps_c = psA.tile([128, E], F32, tag="psg")
                for tp in range(mt + 1):
                    nc.tensor.matmul(
                        ps_c[:],
                        ones_c[:] if tp < mt else ltri_c[:],
                        sel_all[:, tp, :],
                        start=(tp == 0),
                        stop=(tp == mt),
                    )
                slott = rt.tile([128, E], F32, tag="slott")
                nc.vector.tensor_tensor(slott[:], ps_c[:], ecrow_c[:], OP.add)
                tmp = rt.tile([128, E], F32, tag="tmpslot")
                nc.vector.tensor_tensor(tmp[:], slott[:], is1_all[:, mt, :], OP.mult)
                nc.vector.tensor_reduce(slot1[:, mt : mt + 1], tmp[:], axis=AX.X, op=OP.add)
                nc.vector.tensor_tensor(tmp[:], slott[:], is2_all[:, mt, :], OP.mult)
                nc.vector.tensor_reduce(slot2[:, mt : mt + 1], tmp[:], axis=AX.X, op=OP.add)

            # ---- build slot table: scatter token ids (+1) into list slots ----
            vals = meta.tile([128, NT, 64], F32)
            nc.gpsimd.memset(vals[:], 0.0)
            nc.vector.tensor_copy(vals[:, :, 0], tokid_c[:])

            si16 = rt.tile([128, NT], I16, tag="si16")
            sidx = meta.tile([128, ntok // 16], I16)
            nc.gpsimd.memset(sidx[:], 0)
            for r, slotf in ((0, slot1), (1, slot2)):
                nc.vector.tensor_copy(si16[:], slotf[:])
                nc.sync.dma_start(
                    out=sbounce[r].rearrange("(t p) -> p t", p=128), in_=si16[:]
                )
                for kk in range(8):  # replicate across the 8 gpsimd cores
                    nc.sync.dma_start(
                        out=sidx[16 * kk : 16 * (kk + 1), :],
                        in_=sbounce[r].rearrange("(i p) -> p i", p=16),
                    )
                nc.gpsimd.dma_scatter_add(
                    listbuf,
                    vals[:],
                    sidx[:],
                    ntok,
                    ntok_reg,
                    64,
                )

            # ---- read back per-expert token lists -> gather/scatter indices ----
            lists_f = rt.tile([128, E, CI], F32, tag="listsf")
            for kk in range(8):  # replicate across the 8 gpsimd cores
                nc.sync.dma_start(
                    out=lists_f[16 * kk : 16 * (kk + 1), :, :],
                    in_=listbuf.rearrange("(e i p) c -> p e i c", p=16, i=CI)[:, :, :, 0:1],
                )
            vm1 = rt.tile([128, E, CI], F32, tag="vm1")
            nc.vector.tensor_scalar_add(vm1[:], lists_f[:], -1.0)
            isneg = rt.tile([128, E, CI], F32, tag="isneg")
            nc.vector.tensor_scalar(isneg[:], vm1[:], 0.0, None, op0=OP.is_lt)
            gidx_f = rt.tile([128, E, CI], F32, tag="gidxf")
            nc.vector.scalar_tensor_tensor(
                gidx_f[:], isneg[:], float(ntok + 1), vm1[:],
                op0=OP.mult, op1=OP.add,
            )
            nc.vector.tensor_copy(gidx[:], gidx_f[:])



        # ---- phase B: shared expert (dense over the core's tokens) ----
        with (
            tc.tile_pool(name="shp", bufs=1) as shp,
            tc.tile_pool(name="sw1p", bufs=2) as sw1p,
            tc.tile_pool(name="psB", bufs=2, space="PSUM") as psB,
        ):
            xtb = shp.tile([128, KD, ntok], BF)
            nc.sync.dma_start(out=xtb[:], in_=xTb.rearrange("(k p) n -> p k n", p=128))
            hts = shp.tile([128, KH, ntok], BF)
            ns_w = min(512, ntok)
            NS = ntok // ns_w
            for mh in range(KH):
                s1b = sw1p.tile([128, KD, 128], BF, tag="s1b")
                nc.sync.dma_start(out=s1b[:], in_=sw1[mh])
                for ns in range(NS):
                    ps = psB.tile([128, ns_w], F32, tag="ps2")
                    for k in range(KD):
                        nc.tensor.matmul(
                            ps[:],
                            s1b[:, k, :],
                            xtb[:, k, ns * ns_w : (ns + 1) * ns_w],
                            start=(k == 0),
                            stop=(k == KD - 1),
                        )
                    nc.scalar.activation(
                        hts[:, mh, ns * ns_w : (ns + 1) * ns_w], ps[:], AF.Silu
                    )
            sw2b = shp.tile([128, KH, D], BF)
            for kh in range(KH):
                nc.sync.dma_start(out=sw2b[:, kh, :], in_=sw2[kh * 128 : (kh + 1) * 128, :])
            for mt in range(NT):
                for nt in range(D // 512):
                    ps = psB.tile([128, 512], F32, tag="ps2")
                    for kh in range(KH):
                        nc.tensor.matmul(
                            ps[:],
                            hts[:, kh, mt * 128 : (mt + 1) * 128],
                            sw2b[:, kh, nt * 512 : (nt + 1) * 512],
                            start=(kh == 0),
                            stop=(kh == KH - 1),
                        )
                    ys = sw1p.tile([128, 512], F32, tag="ys")
                    nc.vector.tensor_copy(ys[:], ps[:])
                    nc.sync.dma_start(
                        out=out[mt * 128 : (mt + 1) * 128, nt * 512 : (nt + 1) * 512],
                        in_=ys[:],
                    )

        # ---- phase C: routed experts ----
        with (
            tc.tile_pool(name="xg", bufs=3) as xgp,
            tc.tile_pool(name="wgp", bufs=2) as wgp,
            tc.tile_pool(name="w1p", bufs=4) as w1p,
            tc.tile_pool(name="w2p", bufs=2) as w2p,
            tc.tile_pool(name="hep", bufs=2) as hep,
            tc.tile_pool(name="yp", bufs=2) as yp,
            tc.tile_pool(name="psM1", bufs=2, space="PSUM") as psM1,
            tc.tile_pool(name="psM2", bufs=2, space="PSUM") as psM2,
        ):
            def issue_gathers(e):
                eidx = gidx[:, e, :]
                xg = xgp.tile([128, KD, cap], BF, tag="xg", name=f"xg{e}")
                nc.gpsimd.dma_gather(
                    xg[:], xr, eidx, cap, cap_reg, D, transpose=True
                )
                wg = wgp.tile([128, MT, 64], F32, tag="wg", name=f"wg{e}")
                nc.gpsimd.dma_gather(
                    wg[:], combine, eidx, cap, cap_reg, 64, transpose=False
                )
                return xg, wg

            nxt = issue_gathers(0)
            for e in range(E):
                xg, wg = nxt
                if e + 1 < E:
                    # prefetch next expert's gathers ahead of this expert's
                    # scatter in the SWDGE queue
                    nxt = issue_gathers(e + 1)
                w2b = w2p.tile([128, KH, D], BF, tag="w2b")
                for kh in range(KH):
                    nc.sync.dma_start(
                        out=w2b[:, kh, :], in_=w2[e, kh * 128 : (kh + 1) * 128, :]
                    )
                hte = hep.tile([128, KH, cap], BF, tag="hte")
                for mh in range(KH):
                    w1b = w1p.tile([128, KD, 128], BF, tag="w1b")
                    nc.sync.dma_start(out=w1b[:], in_=w1[e, mh])
                    ps1 = psM1.tile([128, cap], F32, tag="ps1")
                    for k in range(KD):
                        nc.tensor.matmul(
                            ps1[:],
                            w1b[:, k, :],
                            xg[:, k, :],
                            start=(k == 0),
                            stop=(k == KD - 1),
                        )
                    nc.scalar.activation(hte[:, mh, :], ps1[:], AF.Silu)
                ye = yp.tile([128, MT, D], F32, tag="ye")
                for mt in range(MT):
                    for nt in range(D // 512):
                        ps2 = psM2.tile([128, 512], F32, tag="ps2")
                        for kh in range(KH):
                            nc.tensor.matmul(
                                ps2[:],
                                hte[:, kh, mt * 128 : (mt + 1) * 128],
                                w2b[:, kh, nt * 512 : (nt + 1) * 512],
                                start=(kh == 0),
                                stop=(kh == KH - 1),
                            )
                        nc.vector.tensor_scalar(
                            ye[:, mt, nt * 512 : (nt + 1) * 512],
                            ps2[:],
                            wg[:, mt, e : e + 1],
                            None,
                            op0=OP.mult,
                        )
                nc.gpsimd.dma_scatter_add(
                    out, ye[:], gidx[:, e, :], cap, cap_reg, D
                )
        ctx_rt.release()


# ---------------------------------------------------------------------------
# host side
# ---------------------------------------------------------------------------

def _consts(cap, ntok=NTOK):
    ltri = np.tril(np.ones((128, 128), np.float32), -1).T.copy()  # [k, m]: 1 if k < m
    ones = np.ones((128, 128), np.float32)
    ecrow = np.tile((np.arange(E, dtype=np.float32) * cap)[None, :], (128, 1))
    iota = np.arange(128, dtype=np.float32)[:, None].copy()
    nt = ntok // 128
    tokid = (np.arange(nt, dtype=np.float32)[None, :] * 128
             + np.arange(128, dtype=np.float32)[:, None] + 1.0).copy()
    return ltri, ones, ecrow, iota, tokid


def _host_prep(x, gate_w, w1, w2, sw1, sw2, ntok=NTOK, cap=CAP):
    """Build the 8 per-core input maps from the full inputs."""
    xf = np.ascontiguousarray(x.reshape(-1, D).astype(np.float32))
    gwT = np.ascontiguousarray(gate_w.astype(np.float32).T)
    kh, kd = H // 128, D // 128
    w1b = np.ascontiguousarray(
        w1.astype(BF16).reshape(E, kd, 128, kh, 128).transpose(0, 3, 2, 1, 4)
    )  # [E, KH, kp, KD, hp]
    w2b = np.ascontiguousarray(w2.astype(BF16))
    sw1b = np.ascontiguousarray(
        sw1[0].astype(BF16).reshape(kd, 128, kh, 128).transpose(2, 1, 0, 3)
    )  # [KH, kp, KD, hp]
    sw2b = np.ascontiguousarray(sw2[0].astype(BF16))
    ltri, ones, ecrow, iota, tokid = _consts(cap, ntok)
    maps = []
    for c in range(NCORES):
        xs = xf[c * ntok : (c + 1) * ntok]
        xsT = np.ascontiguousarray(xs.T)
        xr = np.zeros((ntok + 1, D), BF16)
        xr[:ntok] = xs.astype(BF16)
        maps.append(
            {
                "xT": xsT,
                "xTb": xsT.astype(BF16),
                "xr": xr,
                "gwT": gwT,
                "w1": w1b,
                "w2": w2b,
                "sw1": sw1b,
                "sw2": sw2b,
                "ltri": ltri,
                "ones": ones,
                "ecrow": ecrow,
                "iota": iota,
                "tokid": tokid,
            }
        )
    return maps


_BUILT = {}


def build_nc(ntok=NTOK, cap=CAP, dbg=False):
    nc = bacc.Bacc(
        "TRN2",
        target_bir_lowering=False,
        debug=False,
        num_devices=NCORES,
        num_swdge_queues=3,
    )
    specs = {
        "xT": ([D, ntok], F32),
        "xTb": ([D, ntok], BF),
        "xr": ([ntok + 1, D], BF),
        "gwT": ([D, E], F32),
        "w1": ([E, H // 128, 128, D // 128, 128], BF),
        "w2": ([E, H, D], BF),
        "sw1": ([H // 128, 128, D // 128, 128], BF),
        "sw2": ([H, D], BF),
        "ltri": ([128, 128], F32),
        "ones": ([128, 128], F32),
        "ecrow": ([128, E], F32),
        "iota": ([128, 1], F32),
        "tokid": ([128, ntok // 128], F32),
    }
    ins = {k: nc.dram_tensor(k, shp, dt, kind="ExternalInput").ap() for k, (shp, dt) in specs.items()}
    outs = {"out": nc.dram_tensor("out", [ntok + 1, D], F32, kind="ExternalOutput").ap()}
    if dbg:
        outs["combine"] = nc.dram_tensor("combine", [ntok + 1, 64], F32, kind="ExternalOutput").ap()
        outs["listbuf"] = nc.dram_tensor("listbuf", [E * cap, 64], F32, kind="ExternalOutput").ap()
    with tile.TileContext(nc) as tc:
        emit_moe(tc, ins, outs, ntok=ntok, cap=cap, dbg=dbg)
    nc.compile()
    return nc


def kernel(x, gate_w, w1, w2, sw1, sw2):
    x, gate_w, w1, w2, sw1, sw2 = (
        np.asarray(a) for a in (x, gate_w, w1, w2, sw1, sw2)
    )
    maps = _host_prep(x, gate_w, w1, w2, sw1, sw2)
    if "nc" not in _BUILT:
        _BUILT["nc"] = build_nc()
    nc = _BUILT["nc"]
    res = run_bass_kernel_spmd(nc, maps, list(range(NCORES)))
    outs = [res.results[c]["out"][:NTOK] for c in range(NCORES)]
    return np.concatenate(outs, axis=0).reshape(B, T, D).astype(np.float32)


def timed_run(inputs, iters=6):
    """Time device execution: jit once, inputs resident on device, min over
    repeats (in ns). Mirrors bass2jax.run_bass_via_pjrt's multi-core path."""
    import time

    import jax
    import concourse.mybir as mb
    from jax.experimental.shard_map import shard_map
    from jax.sharding import Mesh, NamedSharding, PartitionSpec
    from concourse import bass2jax

    bass2jax.install_neuronx_cc_hook()
    maps = _host_prep(**inputs)
    if "nc" not in _BUILT:
        _BUILT["nc"] = build_nc()
    nc = _BUILT["nc"]

    pname = nc.partition_id_tensor.name if nc.partition_id_tensor else None
    in_names, out_names, out_avals = [], [], []
    for alloc in nc.m.functions[0].allocations:
        if not isinstance(alloc, mb.MemoryLocationSet):
            continue
        name = alloc.memorylocations[0].name
        if alloc.kind == "ExternalInput":
            if name != pname:
                in_names.append(name)
        elif alloc.kind == "ExternalOutput":
            out_names.append(name)
            out_avals.append(
                jax.core.ShapedArray(tuple(alloc.tensor_shape), mb.dt.np(alloc.dtype))
            )
    all_in_names = list(in_names) + ([pname] if pname else [])

    def _body(*args):
        operands = list(args)
        if pname:
            operands.append(bass2jax.partition_id_tensor())
        outs = bass2jax._bass_exec_p.bind(
            *operands,
            out_avals=tuple(out_avals),
            in_names=tuple(all_in_names),
            out_names=tuple(out_names),
            lowering_input_output_aliases=(),
            sim_require_finite=True,
            sim_require_nnan=True,
            nc=nc,
        )
        return tuple(outs)

    devices = jax.devices()[:NCORES]
    mesh = Mesh(np.asarray(devices), ("core",))
    fn = jax.jit(
        shard_map(
            _body,
            mesh=mesh,
            in_specs=(PartitionSpec("core"),) * len(in_names),
            out_specs=(PartitionSpec("core"),) * len(out_names),
            check_rep=False,
        ),
        keep_unused=True,
    )
    sharding = NamedSharding(mesh, PartitionSpec("core"))
    dev_args = [
        jax.device_put(
            np.concatenate([np.asarray(maps[c][n]) for c in range(NCORES)], axis=0),
            sharding,
        )
        for n in in_names
    ]
    jax.block_until_ready(dev_args)
    # warmup (compile)
    jax.block_until_ready(fn(*dev_args))
    best = float("inf")
    for _ in range(iters):
        t0 = time.perf_counter()
        jax.block_until_ready(fn(*dev_args))
        best = min(best, time.perf_counter() - t0)
    return best * 1e9
